# revision 1
# baseline (speedup 1.0000x reference)
"""GEMV kernel for Trainium2: out = x @ W.T + b, sharded over 8 NeuronCores.

Shapes (hardcoded): x [1, 147456] f32, W [1000, 147456] f32, b [1000] f32.
Sharding: W's 1000 output classes split 8 ways (125 rows/core); each core
computes its 125 dot products locally, outputs concatenate host-side.

Per-core strategy (memory-bound: 73.7 MB of W per core at ~360 GB/s):
  - View k = k2*1152 + k1, so each W row [147456] maps to an SBUF tile
    [128 (k2, partitions), 1152 (k1, free)] with 4.6 KB contiguous per
    partition -> DMA at line rate.
  - x is reshaped the same way and stays resident in SBUF.
  - One fused DVE scalar_tensor_tensor per row m: out = (W*1.0)*x with
    accum_out = per-partition sum, i.e. multiply + free-dim reduce in a
    single pass (~1.05 cycles per element-per-partition), accumulating
    per-k2 partial sums into column m of a [128, 125] accumulator.
    (tensor_tensor_reduce would be the natural op but its opcode crashes
    the exec unit on this HW/compiler; TensorScalarPtr works.)
  - A single PE matmul against a ones vector reduces the accumulator across
    partitions -> [1, 125]; add bias; DMA out.
"""

import numpy as np

import concourse.bacc as bacc
import concourse.mybir as mybir
import concourse.tile as tile
from concourse.bass_utils import run_bass_kernel_spmd

N_CORES = 8
N_CLASSES = 1000
N_IN = 147456
P = 128                      # partitions (k2)
K1 = N_IN // P               # 1152 free elements per partition
M = N_CLASSES // N_CORES     # 125 rows per core
MT = 5                       # W rows per DMA chunk (25 chunks of 2.95 MB)

_prog_cache = {}


def _build_program():
    if "nc" in _prog_cache:
        return _prog_cache["nc"]

    nc = bacc.Bacc("TRN2", target_bir_lowering=False, debug=False, num_devices=N_CORES)
    f32 = mybir.dt.float32
    x_d = nc.dram_tensor("x", [P, K1], f32, kind="ExternalInput")
    w_d = nc.dram_tensor("W", [M, P, K1], f32, kind="ExternalInput")
    b_d = nc.dram_tensor("b", [1, M], f32, kind="ExternalInput")
    o_d = nc.dram_tensor("out", [1, M], f32, kind="ExternalOutput")
    ones_d = nc.inline_tensor(np.ones((P, 1), np.float32), "ones_const")

    with tile.TileContext(nc) as tc:
        with (
            tc.tile_pool(name="xpool", bufs=1) as xpool,
            tc.tile_pool(name="wpool", bufs=3) as wpool,
            tc.tile_pool(name="misc", bufs=1) as misc,
            tc.tile_pool(name="psum", bufs=1, space="PSUM") as psum_pool,
        ):
            x_t = xpool.tile([P, K1], f32)
            nc.sync.dma_start(x_t[:], x_d[:])
            ones_t = misc.tile([P, 1], f32)
            nc.sync.dma_start(ones_t[:], ones_d[:])
            b_t = misc.tile([1, M], f32)
            nc.sync.dma_start(b_t[:], b_d[:])
            acc_t = misc.tile([P, M], f32)

            for c in range(0, M, MT):
                mt = min(MT, M - c)
                w_t = wpool.tile([P, MT, K1], f32, tag="w")
                nc.sync.dma_start(
                    w_t[:, :mt, :], w_d[c : c + mt].rearrange("m p k -> p m k")
                )
                for j in range(mt):
                    m = c + j
                    dummy_t = wpool.tile([P, K1], f32, tag="s")
                    nc.vector.scalar_tensor_tensor(
                        out=dummy_t[:],
                        in0=w_t[:, j, :],
                        scalar=1.0,
                        in1=x_t[:],
                        op0=mybir.AluOpType.mult,
                        op1=mybir.AluOpType.mult,
                        accum_out=acc_t[:, m : m + 1],
                    )

            ps = psum_pool.tile([1, M], f32)
            nc.tensor.matmul(ps[:], ones_t[:], acc_t[:], start=True, stop=True)
            out_t = misc.tile([1, M], f32)
            nc.vector.tensor_add(out_t[:], ps[:], b_t[:])
            nc.sync.dma_start(o_d[:], out_t[:])

    nc.finalize()
    _prog_cache["nc"] = nc
    return nc


def _in_maps(x, W, b):
    x128 = np.ascontiguousarray(np.asarray(x).reshape(P, K1), dtype=np.float32)
    in_maps = []
    for c in range(N_CORES):
        sl = slice(c * M, (c + 1) * M)
        in_maps.append(
            {
                "x": x128,
                "W": np.ascontiguousarray(W[sl].reshape(M, P, K1), dtype=np.float32),
                "b": np.ascontiguousarray(b[sl].reshape(1, M), dtype=np.float32),
            }
        )
    return in_maps


def _run(x, W, b, trace=False, **kwargs):
    nc = _build_program()
    in_maps = _in_maps(x, W, b)
    return run_bass_kernel_spmd(nc, in_maps, list(range(N_CORES)), trace=trace, **kwargs)


def kernel(x, W, b):
    res = _run(x, W, b)
    outs = [r["out"].reshape(1, M) for r in res.results]
    return np.concatenate(outs, axis=1).astype(np.float32)



# revision 2
# speedup vs baseline: 15.0995x; 15.0995x over previous
"""GEMV kernel for Trainium2: out = x @ W.T + b, sharded over 8 NeuronCores.

Shapes (hardcoded): x [1, 147456] f32, W [1000, 147456] f32, b [1000] f32.
Sharding: W's 1000 output classes split 8 ways (125 rows/core, padded to
128); each core computes its rows' dot products locally, outputs
concatenate host-side (no device collective).

Per-core strategy (memory-bound):
  - W rows are ternary {-1,0,1}: exactly representable in bf16, halving
    HBM traffic (73.7 MB -> 36.9 MB per core). x is cast to bf16 too
    (rel err ~1e-3, far under the 2e-2 gate).
  - Padded W [128, 147456] is viewed as 8 chunks of 16 rows; each chunk
    is a fully contiguous 4.7 MB DMA into an SBUF tile [128, 18432]
    (36.9 KB contiguous per partition -> line-rate descriptors).
    Partition p of chunk j holds row 16j + p//8, segment p%8.
  - x is pre-replicated host-side to the matching layout xr[p] =
    x[(p%8)*18432 : (p%8+1)*18432] and stays resident in SBUF.
  - One DVE scalar_tensor_tensor per chunk: (W*1.0)*x with accum_out =
    per-partition sum into acc[:, j]. All-bf16 operands keep both SBUF
    read ports fed -> full DVE rate; accumulation is fp32 internally.
  - One PE matmul vs a [128,16] group-indicator reduces acc over the
    8-partition groups -> psum [8,16] where flat index j*16+i = row
    16j+i; add bias; DMA out. Host takes flat[:125] per core.
  - loop_n repeats the whole computation in-NEFF (back-to-back GEMV
    executions) so per-dispatch overhead amortizes when timing.
"""

import numpy as np
import ml_dtypes

import concourse.bacc as bacc
import concourse.mybir as mybir
import concourse.tile as tile
from concourse.bass_utils import run_bass_kernel_spmd

N_CORES = 8
N_CLASSES = 1000
N_IN = 147456
P = 128                      # SBUF partitions
M = N_CLASSES // N_CORES     # 125 real rows per core
MP = 128                     # padded rows per core
RPC = 16                     # rows per chunk
NCH = MP // RPC              # 8 chunks
F = N_IN // (P // RPC)       # 18432 free elements per partition per chunk
SEG = P // RPC               # 8 partitions (x segments) per row

_prog_cache = {}


def _build_program(loop_n=1):
    if loop_n in _prog_cache:
        return _prog_cache[loop_n]

    nc = bacc.Bacc("TRN2", target_bir_lowering=False, debug=False, num_devices=N_CORES)
    f32 = mybir.dt.float32
    bf16 = mybir.dt.bfloat16
    x_d = nc.dram_tensor("x", [P, F], bf16, kind="ExternalInput")
    w_d = nc.dram_tensor("W", [NCH, P, F], bf16, kind="ExternalInput")
    b_d = nc.dram_tensor("b", [NCH, RPC], f32, kind="ExternalInput")
    o_d = nc.dram_tensor("out", [NCH, RPC], f32, kind="ExternalOutput")
    # ind[p, i] = 1.0 where p//SEG == i: sums the SEG partition-partials
    # of each row. psum[j, i] = dot(row 16j+i).
    ind_np = np.zeros((P, RPC), np.float32)
    ind_np[np.arange(P), np.arange(P) // SEG] = 1.0
    ind_d = nc.inline_tensor(ind_np, "ind_const")

    with tile.TileContext(nc) as tc:
        with (
            tc.tile_pool(name="xpool", bufs=1) as xpool,
            tc.tile_pool(name="wpool", bufs=3) as wpool,
            tc.tile_pool(name="dpool", bufs=1) as dpool,
            tc.tile_pool(name="accpool", bufs=2) as accpool,
            tc.tile_pool(name="misc", bufs=1) as misc,
            tc.tile_pool(name="opool", bufs=2) as opool,
            tc.tile_pool(name="psum", bufs=2, space="PSUM") as psum_pool,
        ):
            x_t = xpool.tile([P, F], bf16)
            nc.sync.dma_start(x_t[:], x_d[:])
            ind_t = misc.tile([P, RPC], f32)
            nc.sync.dma_start(ind_t[:], ind_d[:])
            b_t = misc.tile([NCH, RPC], f32)
            nc.sync.dma_start(b_t[:], b_d[:])
            dummy_t = dpool.tile([P, F], bf16)

            for _ in range(loop_n):
                acc_t = accpool.tile([P, NCH], f32, tag="acc")
                for c in range(NCH):
                    w_t = wpool.tile([P, F], bf16, tag="w")
                    nc.sync.dma_start(w_t[:], w_d[c])
                    nc.vector.scalar_tensor_tensor(
                        out=dummy_t[:],
                        in0=w_t[:],
                        scalar=1.0,
                        in1=x_t[:],
                        op0=mybir.AluOpType.mult,
                        op1=mybir.AluOpType.mult,
                        accum_out=acc_t[:, c : c + 1],
                    )
                ps = psum_pool.tile([NCH, RPC], f32, tag="ps")
                nc.tensor.matmul(ps[:], acc_t[:], ind_t[:], start=True, stop=True)
                out_t = opool.tile([NCH, RPC], f32, tag="o")
                nc.vector.tensor_add(out_t[:], ps[:], b_t[:])
                nc.sync.dma_start(o_d[:], out_t[:])

    nc.finalize()
    _prog_cache[loop_n] = nc
    return nc


def _in_maps(x, W, b):
    bf = ml_dtypes.bfloat16
    x_flat = np.asarray(x, dtype=np.float32).reshape(N_IN)
    xr = np.tile(x_flat.reshape(SEG, F), (RPC, 1)).astype(bf)
    in_maps = []
    for c in range(N_CORES):
        sl = slice(c * M, (c + 1) * M)
        wp = np.zeros((MP, N_IN), dtype=bf)
        wp[:M] = np.asarray(W[sl], dtype=np.float32)
        bp = np.zeros(MP, dtype=np.float32)
        bp[:M] = np.asarray(b[sl], dtype=np.float32)
        in_maps.append(
            {
                "x": xr,
                "W": np.ascontiguousarray(wp.reshape(NCH, P, F)),
                "b": bp.reshape(NCH, RPC),
            }
        )
    return in_maps


def _run(x, W, b, trace=False, loop_n=1, **kwargs):
    nc = _build_program(loop_n)
    in_maps = _in_maps(x, W, b)
    return run_bass_kernel_spmd(nc, in_maps, list(range(N_CORES)), trace=trace, **kwargs)


def kernel(x, W, b):
    res = _run(x, W, b)
    outs = [r["out"].reshape(-1)[:M].reshape(1, M) for r in res.results]
    return np.concatenate(outs, axis=1).astype(np.float32)
